# revision 1
# baseline (speedup 1.0000x reference)
"""Block-FFT circulant matmul (BlockFFTDirectPrior) as a Trainium2 Bass kernel.

Math: out = ifft( einsum('bjf,ijf->bif', fft(x_blocks), conj(W_full)) ).real
with 64x64 blocks of size 256, batch 2048.

Everything is real-matmul based (no complex arithmetic, no FFT butterflies):
  stage 1: per input block j, spectrum = x_j @ R            (DFT as matmul)
  stage 2: per frequency slot s, mix blocks j -> i with a 128x128 real
           matrix G_s built from W_real/W_imag (re/im packed)
  stage 3: per output block i, time = spectrum_i @ Rinv     (IDFT as matmul)

Spectrum packing (256 real values per block): half A = Re f=0..127,
half B = [Re f=128, Im f=1..127].  Slot s pairs (A[s], B[s]): slot 0
carries DC/Nyquist (both real), slots 1..127 carry complex bin f=s.

Between stages the partition axis must rotate (spec -> block -> spec).
Scatter-DMAs are descriptor-bound on TRN2, so both permutes are done as
PE transposes: matmul(lhsT=data, rhs=identity) -> data^T in PSUM.

Sharding: data-parallel over batch across 8 NeuronCores (256 rows each),
2 passes of 128 rows per core to fit SBUF.  All matmul operands are bf16
(PSUM accumulation is fp32); output is fp32.
"""

import numpy as np
import ml_dtypes

import concourse.bass as bass
import concourse.mybir as mybir
from concourse import bacc
from concourse.tile import TileContext
from concourse.bass_utils import run_bass_kernel_spmd

B, KIN, KOUT, BLOCK = 2048, 64, 64, 256
NCORES = 8
BC = B // NCORES            # 256 batch rows per core
NPASS = 2
PB = BC // NPASS            # 128 batch rows per pass

F32 = mybir.dt.float32
BF16 = mybir.dt.bfloat16
U32 = mybir.dt.uint32
NPBF16 = ml_dtypes.bfloat16

_NC_CACHE = {}


def _build_consts():
    """DFT / inverse-DFT matrices + identity, bf16, kernel layouts."""
    t = np.arange(BLOCK)
    f = np.arange(128)
    ang = 2.0 * np.pi * np.outer(t, f) / BLOCK          # [t, f]
    RA = np.cos(ang)                                    # re f=0..127
    RB = -np.sin(ang)                                   # im f=1..127
    RB[:, 0] = np.cos(np.pi * t)                        # re f=128 in col 0
    R = np.zeros((2, 2, 128, 128), dtype=NPBF16)        # [h, kt, t(128), m]
    for kt in range(2):
        R[0, kt] = RA[kt * 128:(kt + 1) * 128, :].astype(NPBF16)
        R[1, kt] = RB[kt * 128:(kt + 1) * 128, :].astype(NPBF16)

    s = np.arange(128)
    tp = np.arange(BLOCK)
    angi = 2.0 * np.pi * np.outer(s, tp) / BLOCK        # [s, t']
    w = np.full((128, 1), 2.0 / BLOCK)
    w[0] = 1.0 / BLOCK
    RiA = w * np.cos(angi)
    RiB = -(2.0 / BLOCK) * np.sin(angi)
    RiB[0, :] = (1.0 / BLOCK) * np.cos(np.pi * tp)      # Nyquist (real) term
    Ri = np.stack([RiA, RiB]).astype(NPBF16)            # [2, 128, 256]

    ident = np.eye(128, dtype=NPBF16)
    return R, Ri, ident


def _build_g(Wr, Wi):
    """Stage-2 mixing matrices, layout [k=(h*64+j), s, m=(re_i|im_i)], bf16."""
    G = np.zeros((128, 128, 128), dtype=np.float32)     # [s, k, m]
    G[0, :64, :64] = Wr[:, :, 0].T
    G[0, 64:, 64:] = Wr[:, :, 128].T
    WrT = np.transpose(Wr, (2, 1, 0))                   # [f, j, i]
    WiT = np.transpose(Wi, (2, 1, 0))
    G[1:, :64, :64] = WrT[1:128]
    G[1:, :64, 64:] = -WiT[1:128]
    G[1:, 64:, :64] = WiT[1:128]
    G[1:, 64:, 64:] = WrT[1:128]
    return np.ascontiguousarray(G.transpose(1, 0, 2)).astype(NPBF16)


def _build_nc():
    nc = bacc.Bacc("TRN2", target_bir_lowering=False, debug=False)
    # xP layout [pass, t(256), b(128), j(64)]
    xP = nc.dram_tensor("xP", [NPASS, BLOCK, PB, KIN], BF16, kind="ExternalInput")
    Gt = nc.dram_tensor("G", [128, 128, 128], BF16, kind="ExternalInput")
    Rt = nc.dram_tensor("R", [2, 2, 128, 128], BF16, kind="ExternalInput")
    Rit = nc.dram_tensor("Ri", [2, 128, 256], BF16, kind="ExternalInput")
    It = nc.dram_tensor("Ident", [128, 128], BF16, kind="ExternalInput")
    Y = nc.dram_tensor("Y", [BC, KOUT * BLOCK], F32, kind="ExternalOutput")

    def copy_eng(k):
        return nc.vector.tensor_copy if k % 2 == 0 else nc.scalar.copy

    with TileContext(nc) as tc:
        with (
            tc.tile_pool(name="const", bufs=1) as cpool,
            tc.tile_pool(name="big", bufs=1) as bigpool,
            tc.tile_pool(name="work", bufs=2) as wpool,
            tc.tile_pool(name="ps", bufs=4, space="PSUM") as pspool,
        ):
            Rsb = cpool.tile([128, 4 * 128], BF16)
            for h in range(2):
                for kt in range(2):
                    nc.sync.dma_start(
                        Rsb[:, (h * 2 + kt) * 128:(h * 2 + kt + 1) * 128],
                        Rt.ap()[h, kt],
                    )
            Risb = cpool.tile([128, 512], BF16)
            for h in range(2):
                nc.sync.dma_start(Risb[:, h * 256:(h + 1) * 256], Rit.ap()[h])
            Isb = cpool.tile([128, 128], BF16)
            nc.sync.dma_start(Isb[:, :], It.ap())

            for p in range(NPASS):
                # ---------------- stage 1: DFT per block ----------------
                # xk[kt] [t(128), (b,j)]; out1 [s, (b, hj)]
                xk = []
                for kt in range(2):
                    xt = bigpool.tile([128, PB * KIN], BF16, tag=f"xk{kt}",
                                      name=f"xk{kt}")
                    for q in range(4):
                        nc.sync.dma_start(
                            xt[:, q * 2048:(q + 1) * 2048],
                            xP.ap()[p, kt * 128:(kt + 1) * 128,
                                    q * 32:(q + 1) * 32],
                        )
                    xk.append(xt)
                # out1 layout [s, (b, hj)]: col = b*128 + h*64 + j
                out1 = bigpool.tile([128, 2 * PB * KIN], BF16, tag="out1")
                out1v = out1.rearrange("p (b hj) -> p b hj", hj=128)
                nck = 0
                for g2 in range(8):  # 2-bank PSUM tiles, 2 chunks each
                    for h in range(2):
                        ps1 = pspool.tile([128, 1024], F32, tag="ps")
                        for q in range(2):
                            g = g2 * 2 + q
                            nc.tensor.matmul(
                                ps1[:, q * 512:(q + 1) * 512],
                                Rsb[:, (h * 2) * 128:(h * 2 + 1) * 128],
                                xk[0][:, g * 512:(g + 1) * 512],
                                start=True, stop=False,
                            )
                            nc.tensor.matmul(
                                ps1[:, q * 512:(q + 1) * 512],
                                Rsb[:, (h * 2 + 1) * 128:(h * 2 + 2) * 128],
                                xk[1][:, g * 512:(g + 1) * 512],
                                start=False, stop=True,
                            )
                        # ps1 [s, (b16, j64)] -> out1 [s, b, h*64+j]
                        copy_eng(nck)(
                            out1v[:, g2 * 16:(g2 + 1) * 16, h * 64:(h + 1) * 64],
                            ps1.rearrange("p (b j) -> p b j", b=16),
                        )
                        nck += 1

                # ---- permute-1: out1 [s,(b,hj)] -> X2 [hj,(b,s)] --------
                X2 = bigpool.tile([128, 128 * PB], BF16, tag="X2")
                X2v = X2.rearrange("p (b s) -> p s b", s=128)  # [p, s, b] view
                for t8 in range(PB // 8):
                    psT = pspool.tile([128, 1024], F32, tag="ps")
                    for q in range(8):
                        bb = t8 * 8 + q
                        nc.tensor.matmul(
                            psT[:, q * 128:(q + 1) * 128],
                            out1[:, bb * 128:(bb + 1) * 128], Isb[:, :],
                            start=True, stop=True,
                        )
                    # psT [hj, (q=b, s)] -> X2 cols, contiguous
                    copy_eng(t8)(X2[:, t8 * 1024:(t8 + 1) * 1024], psT[:, :])

                # ---------------- stage 2: mix blocks per slot -----------
                # O2 [m=(re_i|im_i), (s, b)]
                O2 = bigpool.tile([128, 128 * PB], BF16, tag="O2")
                O2v = O2.rearrange("p (s b) -> p b s", b=PB)   # [p, b, s] view
                gk = None
                for g8 in range(16):  # groups of 8 slots per 2-bank PSUM tile
                    if g8 % 2 == 0:
                        gk = wpool.tile([128, 16 * 128], BF16, tag="gk",
                                        name="gk")
                        nc.gpsimd.dma_start(
                            gk[:, :], Gt.ap()[:, g8 * 8:g8 * 8 + 16]
                        )
                    ps2 = pspool.tile([128, 1024], F32, tag="ps")
                    for q in range(8):
                        s = 8 * g8 + q
                        nc.tensor.matmul(
                            ps2[:, q * 128:(q + 1) * 128],
                            gk[:, (s % 16) * 128:(s % 16 + 1) * 128],
                            X2v[:, s, :],
                            start=True, stop=True,
                        )
                    # ps2 [m, (q=s, b)] -> O2 cols (8*g8+q)*PB + b, contiguous
                    copy_eng(g8)(O2[:, g8 * 1024:(g8 + 1) * 1024], ps2[:, :])

                # ---- permute-2: O2 [m,(s,b)] -> T2 [s,(b,m)] -------------
                T2 = bigpool.tile([128, 128 * PB], BF16, tag="T2")
                T2v = T2.rearrange("p (b m) -> p m b", m=128)  # [p, m, b] view
                for t8 in range(PB // 8):
                    psT = pspool.tile([128, 1024], F32, tag="ps")
                    for q in range(8):
                        bb = t8 * 8 + q
                        nc.tensor.matmul(
                            psT[:, q * 128:(q + 1) * 128],
                            O2v[:, bb, :], Isb[:, :],
                            start=True, stop=True,
                        )
                    # psT [s, (q=b, m)] -> T2 cols, contiguous
                    copy_eng(t8 + 1)(T2[:, t8 * 1024:(t8 + 1) * 1024], psT[:, :])

                # ---------- stage 3: IDFT per output block ---------------
                for g8 in range(8):  # 8 output blocks i -> one [128,2048] store
                    yt = wpool.tile([128, 2048], F32, tag="yt", name="yt")
                    for half in range(2):  # 4 blocks i per 2-bank PSUM tile
                        ps3 = pspool.tile([128, 1024], F32, tag="ps")
                        for q in range(4):
                            i = g8 * 8 + half * 4 + q
                            nc.tensor.matmul(
                                ps3[:, q * 256:(q + 1) * 256],
                                T2v[:, i, :],
                                Risb[:, 0:256], start=True, stop=False,
                            )
                            nc.tensor.matmul(
                                ps3[:, q * 256:(q + 1) * 256],
                                T2v[:, 64 + i, :],
                                Risb[:, 256:512], start=False, stop=True,
                            )
                        copy_eng(half + g8)(
                            yt[:, half * 1024:(half + 1) * 1024], ps3[:, :]
                        )
                    nc.gpsimd.dma_start(
                        Y.ap()[p * PB:(p + 1) * PB, g8 * 2048:(g8 + 1) * 2048],
                        yt[:, :],
                    )
    nc.compile()
    return nc


def _get_nc():
    if "nc" not in _NC_CACHE:
        _NC_CACHE["nc"] = _build_nc()
    return _NC_CACHE["nc"]


def run(x, W_real, W_imag, trace=False):
    x = np.asarray(x, dtype=np.float32)
    Wr = np.asarray(W_real, dtype=np.float32)
    Wi = np.asarray(W_imag, dtype=np.float32)

    nc = _get_nc()
    R, Ri, ident = _build_consts()
    G = _build_g(Wr, Wi)

    in_maps = []
    for c in range(NCORES):
        xc = x[c * BC:(c + 1) * BC]                       # [256, 16384]
        # -> [t, b, j] -> [pass, t(256), b(128), j(64)]
        xcp = xc.reshape(BC, KIN, BLOCK).transpose(2, 0, 1)
        xcp = xcp.reshape(BLOCK, NPASS, PB, KIN).transpose(1, 0, 2, 3)
        in_maps.append({
            "xP": np.ascontiguousarray(xcp).astype(NPBF16),
            "G": G, "R": R, "Ri": Ri, "Ident": ident,
        })
    res = run_bass_kernel_spmd(
        nc, in_maps, core_ids=list(range(NCORES)), trace=trace
    )
    out = np.concatenate([r["Y"] for r in res.results], axis=0)
    return np.ascontiguousarray(out, dtype=np.float32), res


def kernel(x, W_real, W_imag):
    out, _ = run(x, W_real, W_imag)
    return out



# revision 2
# speedup vs baseline: 1.4883x; 1.4883x over previous
"""Block-FFT circulant matmul (BlockFFTDirectPrior) as a Trainium2 Bass kernel.

Math: out = ifft( einsum('bjf,ijf->bif', fft(x_blocks), conj(W_full)) ).real
with 64x64 blocks of size 256, batch 2048.

Everything is real-matmul based (no complex arithmetic, no FFT butterflies):
  stage 1: per input block j, spectrum = x_j @ R            (DFT as matmul)
  stage 2: per frequency slot s, mix blocks j -> i with a 128x128 real
           matrix G_s built from W_real/W_imag (re/im packed)
  stage 3: per output block i, time = spectrum_i @ Rinv     (IDFT as matmul)

Spectrum packing (256 real values per block): half A = Re f=0..127,
half B = [Re f=128, Im f=1..127].  Slot s pairs (A[s], B[s]): slot 0
carries DC/Nyquist (both real), slots 1..127 carry complex bin f=s.

Between stages the partition axis must rotate (spec -> block -> spec).
Both permutes are done as PE transposes: matmul(lhsT=data, rhs=identity).

Layouts are chosen so every PE operand (stationary and moving) is
contiguous in SBUF; the PSUM->SBUF drain copies absorb the reorder
(strided 4B PSUM reads, 16B-chunk SBUF writes):
  out1 [s | b, hj]   (stage-1 out; b-major so T1 lhsT tiles contiguous)
  X2   [hj | s, b]   (s-major so stage-2 moving operand contiguous)
  O2   [m | b, s]    (b-major so T2 lhsT tiles contiguous)
  T2sb [s | m, b]    (m-major so stage-3 lhsT tiles contiguous)

G (4MB bf16) stays resident in SBUF.  Output is written bf16 and upcast
to fp32 on the host during the unshard (halves the output DMA).

Sharding: data-parallel over batch across 8 NeuronCores (256 rows each),
2 passes of 128 rows per core.  All matmul operands are bf16.
"""

import numpy as np
import ml_dtypes

import concourse.bass as bass
import concourse.mybir as mybir
from concourse import bacc
from concourse.tile import TileContext
from concourse.bass_utils import run_bass_kernel_spmd

B, KIN, KOUT, BLOCK = 2048, 64, 64, 256
NCORES = 8
BC = B // NCORES            # 256 batch rows per core
NPASS = 2
PB = BC // NPASS            # 128 batch rows per pass

F32 = mybir.dt.float32
BF16 = mybir.dt.bfloat16
NPBF16 = ml_dtypes.bfloat16

_NC_CACHE = {}


def _build_consts():
    """DFT / inverse-DFT matrices + identity, bf16, kernel layouts."""
    t = np.arange(BLOCK)
    f = np.arange(128)
    ang = 2.0 * np.pi * np.outer(t, f) / BLOCK          # [t, f]
    RA = np.cos(ang)                                    # re f=0..127
    RB = -np.sin(ang)                                   # im f=1..127
    RB[:, 0] = np.cos(np.pi * t)                        # re f=128 in col 0
    R = np.zeros((2, 2, 128, 128), dtype=NPBF16)        # [h, kt, t(128), m]
    for kt in range(2):
        R[0, kt] = RA[kt * 128:(kt + 1) * 128, :].astype(NPBF16)
        R[1, kt] = RB[kt * 128:(kt + 1) * 128, :].astype(NPBF16)

    s = np.arange(128)
    tp = np.arange(BLOCK)
    angi = 2.0 * np.pi * np.outer(s, tp) / BLOCK        # [s, t']
    w = np.full((128, 1), 2.0 / BLOCK)
    w[0] = 1.0 / BLOCK
    RiA = w * np.cos(angi)
    RiB = -(2.0 / BLOCK) * np.sin(angi)
    RiB[0, :] = (1.0 / BLOCK) * np.cos(np.pi * tp)      # Nyquist (real) term
    Ri = np.stack([RiA, RiB]).astype(NPBF16)            # [2, 128, 256]

    ident = np.eye(128, dtype=NPBF16)
    return R, Ri, ident


def _build_g(Wr, Wi):
    """Stage-2 mixing matrices, layout [k=(h*64+j), s, m=(re_i|im_i)], bf16."""
    G = np.zeros((128, 128, 128), dtype=np.float32)     # [s, k, m]
    G[0, :64, :64] = Wr[:, :, 0].T
    G[0, 64:, 64:] = Wr[:, :, 128].T
    WrT = np.transpose(Wr, (2, 1, 0))                   # [f, j, i]
    WiT = np.transpose(Wi, (2, 1, 0))
    G[1:, :64, :64] = WrT[1:128]
    G[1:, :64, 64:] = -WiT[1:128]
    G[1:, 64:, :64] = WiT[1:128]
    G[1:, 64:, 64:] = WrT[1:128]
    return np.ascontiguousarray(G.transpose(1, 0, 2)).astype(NPBF16)


def _build_nc():
    nc = bacc.Bacc("TRN2", target_bir_lowering=False, debug=False)
    # xP layout [pass, t(256), b(128), j(64)]
    xP = nc.dram_tensor("xP", [NPASS, BLOCK, PB, KIN], BF16, kind="ExternalInput")
    Gt = nc.dram_tensor("G", [128, 128, 128], BF16, kind="ExternalInput")
    Rt = nc.dram_tensor("R", [2, 2, 128, 128], BF16, kind="ExternalInput")
    Rit = nc.dram_tensor("Ri", [2, 128, 256], BF16, kind="ExternalInput")
    It = nc.dram_tensor("Ident", [128, 128], BF16, kind="ExternalInput")
    Y = nc.dram_tensor("Y", [BC, KOUT * BLOCK], BF16, kind="ExternalOutput")

    def copy_eng(k):
        return nc.vector.tensor_copy if k % 2 == 0 else nc.scalar.copy

    with TileContext(nc) as tc:
        with (
            tc.tile_pool(name="const", bufs=1) as cpool,
            tc.tile_pool(name="big", bufs=1) as bigpool,
            tc.tile_pool(name="work", bufs=2) as wpool,
            tc.tile_pool(name="ps", bufs=4, space="PSUM") as pspool,
        ):
            Rsb = cpool.tile([128, 4 * 128], BF16)
            for h in range(2):
                for kt in range(2):
                    nc.sync.dma_start(
                        Rsb[:, (h * 2 + kt) * 128:(h * 2 + kt + 1) * 128],
                        Rt.ap()[h, kt],
                    )
            Risb = cpool.tile([128, 512], BF16)
            for h in range(2):
                nc.sync.dma_start(Risb[:, h * 256:(h + 1) * 256], Rit.ap()[h])
            Isb = cpool.tile([128, 128], BF16)
            nc.sync.dma_start(Isb[:, :], It.ap())
            # G resident: [k | s, m], col = s*128 + m
            Gsb = cpool.tile([128, 128 * 128], BF16)
            for c in range(4):
                nc.gpsimd.dma_start(
                    Gsb[:, c * 4096:(c + 1) * 4096],
                    Gt.ap()[:, c * 32:(c + 1) * 32],
                )

            for p in range(NPASS):
                # ---------------- stage 1: DFT per block ----------------
                # xk[kt] [t(128), (b,j)]; out1 [s, (b, hj)]
                xk = [
                    bigpool.tile([128, PB * KIN], BF16, tag=f"xk{kt}",
                                 name=f"xk{kt}")
                    for kt in range(2)
                ]
                for q in range(4):      # b-chunk outer, kt inner: MM order
                    for kt in range(2):
                        nc.sync.dma_start(
                            xk[kt][:, q * 2048:(q + 1) * 2048],
                            xP.ap()[p, kt * 128:(kt + 1) * 128,
                                    q * 32:(q + 1) * 32],
                        )
                # out1 layout [s, (b, hj)]: col = b*128 + h*64 + j
                out1 = bigpool.tile([128, 2 * PB * KIN], BF16, tag="big1")
                out1v = out1.rearrange("p (b hj) -> p b hj", hj=128)
                nck = 0
                for g2 in range(8):  # 2-bank PSUM tiles, 2 chunks each
                    for h in range(2):
                        ps1 = pspool.tile([128, 1024], F32, tag="ps")
                        for q in range(2):
                            g = g2 * 2 + q
                            nc.tensor.matmul(
                                ps1[:, q * 512:(q + 1) * 512],
                                Rsb[:, (h * 2) * 128:(h * 2 + 1) * 128],
                                xk[0][:, g * 512:(g + 1) * 512],
                                start=True, stop=False,
                            )
                            nc.tensor.matmul(
                                ps1[:, q * 512:(q + 1) * 512],
                                Rsb[:, (h * 2 + 1) * 128:(h * 2 + 2) * 128],
                                xk[1][:, g * 512:(g + 1) * 512],
                                start=False, stop=True,
                            )
                        # ps1 [s, (b16, j64)] -> out1 [s, b, h*64+j]
                        copy_eng(nck)(
                            out1v[:, g2 * 16:(g2 + 1) * 16, h * 64:(h + 1) * 64],
                            ps1.rearrange("p (b j) -> p b j", b=16),
                        )
                        nck += 1

                # ---- permute-1: out1 [s,(b,hj)] -> X2 [hj,(s,b)] --------
                X2 = bigpool.tile([128, 128 * PB], BF16, tag="X2")
                X2v = X2.rearrange("p (s b) -> p s b", b=PB)  # [p, s, b] view
                for t8 in range(PB // 8):
                    psT = pspool.tile([128, 1024], F32, tag="ps")
                    for q in range(8):
                        bb = t8 * 8 + q
                        nc.tensor.matmul(
                            psT[:, q * 128:(q + 1) * 128],
                            out1[:, bb * 128:(bb + 1) * 128], Isb[:, :],
                            start=True, stop=True,
                        )
                    # psT [hj, (b8, s128)] -> X2 cols s*PB + b (16B chunks)
                    copy_eng(t8)(
                        X2v[:, :, t8 * 8:(t8 + 1) * 8],
                        psT.rearrange("p (b s) -> p s b", b=8),
                    )

                # ---------------- stage 2: mix blocks per slot -----------
                # O2 [m, (b, s)]: col = b*128 + s  (shares buffer with out1)
                O2 = bigpool.tile([128, PB * 128], BF16, tag="big1")
                O2v = O2.rearrange("p (b s) -> p b s", s=128)
                for g8 in range(16):  # 8 slots per 2-bank PSUM tile
                    ps2 = pspool.tile([128, 1024], F32, tag="ps")
                    for q in range(8):
                        s = 8 * g8 + q
                        nc.tensor.matmul(
                            ps2[:, q * 128:(q + 1) * 128],
                            Gsb[:, s * 128:(s + 1) * 128],
                            X2[:, s * PB:(s + 1) * PB],
                            start=True, stop=True,
                        )
                    # ps2 [m, (s8, b128)] -> O2 cols b*128 + s (16B chunks)
                    copy_eng(g8 + 1)(
                        O2v[:, :, g8 * 8:(g8 + 1) * 8],
                        ps2.rearrange("p (s b) -> p b s", s=8),
                    )

                # ---- permute-2: O2 [m,(b,s)] -> T2 [s,(m,b)] -------------
                T2 = bigpool.tile([128, 128 * PB], BF16, tag="T2")
                T2v = T2.rearrange("p (m b) -> p m b", b=PB)
                for t8 in range(PB // 8):
                    psT = pspool.tile([128, 1024], F32, tag="ps")
                    for q in range(8):
                        bb = t8 * 8 + q
                        nc.tensor.matmul(
                            psT[:, q * 128:(q + 1) * 128],
                            O2[:, bb * 128:(bb + 1) * 128], Isb[:, :],
                            start=True, stop=True,
                        )
                    # psT [s, (b8, m128)] -> T2 cols m*PB + b (16B chunks)
                    copy_eng(t8 + 1)(
                        T2v[:, :, t8 * 8:(t8 + 1) * 8],
                        psT.rearrange("p (b m) -> p m b", b=8),
                    )

                # ---------- stage 3: IDFT per output block ---------------
                for g8 in range(8):  # 8 output blocks i -> one [128,2048] store
                    yt = wpool.tile([128, 2048], BF16, tag="yt", name="yt")
                    for half in range(2):  # 4 blocks i per 2-bank PSUM tile
                        ps3 = pspool.tile([128, 1024], F32, tag="ps")
                        for q in range(4):
                            i = g8 * 8 + half * 4 + q
                            nc.tensor.matmul(
                                ps3[:, q * 256:(q + 1) * 256],
                                T2[:, i * PB:(i + 1) * PB],
                                Risb[:, 0:256], start=True, stop=False,
                            )
                            nc.tensor.matmul(
                                ps3[:, q * 256:(q + 1) * 256],
                                T2[:, (64 + i) * PB:(65 + i) * PB],
                                Risb[:, 256:512], start=False, stop=True,
                            )
                        copy_eng(half + g8)(
                            yt[:, half * 1024:(half + 1) * 1024], ps3[:, :]
                        )
                    nc.gpsimd.dma_start(
                        Y.ap()[p * PB:(p + 1) * PB, g8 * 2048:(g8 + 1) * 2048],
                        yt[:, :],
                    )
    nc.compile()
    return nc


def _get_nc():
    if "nc" not in _NC_CACHE:
        _NC_CACHE["nc"] = _build_nc()
    return _NC_CACHE["nc"]


def run(x, W_real, W_imag, trace=False):
    x = np.asarray(x, dtype=np.float32)
    Wr = np.asarray(W_real, dtype=np.float32)
    Wi = np.asarray(W_imag, dtype=np.float32)

    nc = _get_nc()
    R, Ri, ident = _build_consts()
    G = _build_g(Wr, Wi)

    in_maps = []
    for c in range(NCORES):
        xc = x[c * BC:(c + 1) * BC]                       # [256, 16384]
        # -> [t, b, j] -> [pass, t(256), b(128), j(64)]
        xcp = xc.reshape(BC, KIN, BLOCK).transpose(2, 0, 1)
        xcp = xcp.reshape(BLOCK, NPASS, PB, KIN).transpose(1, 0, 2, 3)
        in_maps.append({
            "xP": np.ascontiguousarray(xcp).astype(NPBF16),
            "G": G, "R": R, "Ri": Ri, "Ident": ident,
        })
    res = run_bass_kernel_spmd(
        nc, in_maps, core_ids=list(range(NCORES)), trace=trace
    )
    out = np.concatenate([r["Y"] for r in res.results], axis=0)
    return np.ascontiguousarray(out.astype(np.float32)), res


def kernel(x, W_real, W_imag):
    out, _ = run(x, W_real, W_imag)
    return out


# revision 6
# speedup vs baseline: 1.6227x; 1.0903x over previous
"""Block-FFT circulant matmul (BlockFFTDirectPrior) as a Trainium2 Bass kernel.

Math: out = ifft( einsum('bjf,ijf->bif', fft(x_blocks), conj(W_full)) ).real
with 64x64 blocks of size 256, batch 2048.

Everything is real-matmul based (no complex arithmetic, no FFT butterflies):
  stage 1: per input block j, spectrum = x_j @ R            (DFT as matmul)
  stage 2: per frequency slot s, mix blocks j -> i with a 128x128 real
           matrix G_s built from W_real/W_imag (re/im packed)
  stage 3: per output block i, time = spectrum_i @ Rinv     (IDFT as matmul)

Spectrum packing (256 real values per block): half A = Re f=0..127,
half B = [Re f=128, Im f=1..127].  Slot s pairs (A[s], B[s]): slot 0
carries DC/Nyquist (both real), slots 1..127 carry complex bin f=s.

Between stages the partition axis must rotate (spec -> block -> spec).
Both permutes are done as PE transposes: matmul(lhsT=data, rhs=identity).

Layouts are chosen so every PE operand (stationary and moving) is
contiguous in SBUF; the PSUM->SBUF drain copies absorb the reorder
(strided 4B PSUM reads, 16B-chunk SBUF writes):
  out1 [s | b, hj]   (stage-1 out; b-major so T1 lhsT tiles contiguous)
  X2   [hj | s, b]   (s-major so stage-2 moving operand contiguous)
  O2   [m | b, s]    (b-major so T2 lhsT tiles contiguous)
  T2sb [s | m, b]    (m-major so stage-3 lhsT tiles contiguous)

G (4MB bf16) stays resident in SBUF.  Output is written bf16 and upcast
to fp32 on the host during the unshard (halves the output DMA).

Sharding: data-parallel over batch across 8 NeuronCores (256 rows each),
2 passes of 128 rows per core.  All matmul operands are bf16.
"""

import numpy as np
import ml_dtypes

import concourse.bass as bass
import concourse.mybir as mybir
from concourse import bacc
from concourse.tile import TileContext
from concourse.bass_utils import run_bass_kernel_spmd

B, KIN, KOUT, BLOCK = 2048, 64, 64, 256
NCORES = 8
BC = B // NCORES            # 256 batch rows per core
NPASS = 2
PB = BC // NPASS            # 128 batch rows per pass

F32 = mybir.dt.float32
BF16 = mybir.dt.bfloat16
NPBF16 = ml_dtypes.bfloat16

_NC_CACHE = {}


def _build_consts():
    """DFT / inverse-DFT matrices + identity, bf16, kernel layouts."""
    t = np.arange(BLOCK)
    f = np.arange(128)
    ang = 2.0 * np.pi * np.outer(t, f) / BLOCK          # [t, f]
    RA = np.cos(ang)                                    # re f=0..127
    RB = -np.sin(ang)                                   # im f=1..127
    RB[:, 0] = np.cos(np.pi * t)                        # re f=128 in col 0
    R = np.zeros((2, 2, 128, 128), dtype=NPBF16)        # [h, kt, t(128), m]
    for kt in range(2):
        R[0, kt] = RA[kt * 128:(kt + 1) * 128, :].astype(NPBF16)
        R[1, kt] = RB[kt * 128:(kt + 1) * 128, :].astype(NPBF16)

    s = np.arange(128)
    tp = np.arange(BLOCK)
    angi = 2.0 * np.pi * np.outer(s, tp) / BLOCK        # [s, t']
    w = np.full((128, 1), 2.0 / BLOCK)
    w[0] = 1.0 / BLOCK
    RiA = w * np.cos(angi)
    RiB = -(2.0 / BLOCK) * np.sin(angi)
    RiB[0, :] = (1.0 / BLOCK) * np.cos(np.pi * tp)      # Nyquist (real) term
    Ri = np.stack([RiA, RiB]).astype(NPBF16)            # [2, 128, 256]

    ident = np.eye(128, dtype=NPBF16)
    # pack for single-DMA loads: R4 [t(128), (h,kt,m)=512], Ri2 [s(128), (h,t')=512]
    R4 = np.ascontiguousarray(R.transpose(2, 0, 1, 3).reshape(128, 512))
    Ri2 = np.ascontiguousarray(Ri.transpose(1, 0, 2).reshape(128, 512))
    return R4, Ri2, ident


def _build_g(Wr, Wi):
    """Stage-2 mixing matrices, layout [k=(h*64+j), s, m=(re_i|im_i)], bf16."""
    G = np.zeros((128, 128, 128), dtype=np.float32)     # [s, k, m]
    G[0, :64, :64] = Wr[:, :, 0].T
    G[0, 64:, 64:] = Wr[:, :, 128].T
    WrT = np.transpose(Wr, (2, 1, 0))                   # [f, j, i]
    WiT = np.transpose(Wi, (2, 1, 0))
    G[1:, :64, :64] = WrT[1:128]
    G[1:, :64, 64:] = -WiT[1:128]
    G[1:, 64:, :64] = WiT[1:128]
    G[1:, 64:, 64:] = WrT[1:128]
    return np.ascontiguousarray(G.transpose(1, 0, 2)).astype(NPBF16)


def _build_nc():
    nc = bacc.Bacc("TRN2", target_bir_lowering=False, debug=False)
    # xP layout [pass, t(256), b(128), j(64)]
    xP = nc.dram_tensor("xP", [NPASS, BLOCK, PB, KIN], BF16, kind="ExternalInput")
    Gt = nc.dram_tensor("G", [128, 128, 128], BF16, kind="ExternalInput")
    Rt = nc.dram_tensor("R", [128, 512], BF16, kind="ExternalInput")
    Rit = nc.dram_tensor("Ri", [128, 512], BF16, kind="ExternalInput")
    It = nc.dram_tensor("Ident", [128, 128], BF16, kind="ExternalInput")
    Y = nc.dram_tensor("Y", [BC, KOUT * BLOCK], BF16, kind="ExternalOutput")

    def copy_eng(k):
        return nc.vector.tensor_copy if k % 2 == 0 else nc.scalar.copy

    with TileContext(nc) as tc:
        with (
            tc.tile_pool(name="const", bufs=1) as cpool,
            tc.tile_pool(name="big", bufs=1) as bigpool,
            tc.tile_pool(name="work", bufs=4) as wpool,
            tc.tile_pool(name="ps", bufs=4, space="PSUM") as pspool,
        ):
            # R first on the sync queue (needed by the first stage-1 MM,
            # tiny), then the pass-0 x chunks follow immediately behind it.
            Rsb = cpool.tile([128, 4 * 128], BF16)
            nc.sync.dma_start(Rsb[:, :], Rt.ap())
            # Ri / I on the scalar HWDGE queue, G on gpsimd SWDGE: keeps the
            # sync queue free for the x input stream.
            Risb = cpool.tile([128, 512], BF16)
            nc.scalar.dma_start(Risb[:, :], Rit.ap())
            Isb = cpool.tile([128, 128], BF16)
            nc.scalar.dma_start(Isb[:, :], It.ap())
            # G resident: [k | s, m], col = s*128 + m
            Gsb = cpool.tile([128, 128 * 128], BF16)
            for c in range(4):
                nc.gpsimd.dma_start(
                    Gsb[:, c * 4096:(c + 1) * 4096],
                    Gt.ap()[:, c * 32:(c + 1) * 32],
                )

            for p in range(NPASS):
                # ---------------- stage 1: DFT per block ----------------
                # xk[kt] [t(128), (b,j)]; out1 [s, (b, hj)]
                xk = [
                    bigpool.tile([128, PB * KIN], BF16, tag=f"xk{kt}",
                                 name=f"xk{kt}")
                    for kt in range(2)
                ]
                for q in range(4):      # b-chunk outer, kt inner: MM order
                    for kt in range(2):
                        nc.sync.dma_start(
                            xk[kt][:, q * 2048:(q + 1) * 2048],
                            xP.ap()[p, kt * 128:(kt + 1) * 128,
                                    q * 32:(q + 1) * 32],
                        )
                # out1 layout [s, (b, hj)]: col = b*128 + h*64 + j
                out1 = bigpool.tile([128, 2 * PB * KIN], BF16, tag="big1")
                out1v = out1.rearrange("p (b hj) -> p b hj", hj=128)
                nck = 0
                for g2 in range(8):  # 2-bank PSUM tiles, 2 chunks each
                    for h in range(2):
                        ps1 = pspool.tile([128, 1024], F32, tag="ps")
                        for q in range(2):
                            g = g2 * 2 + q
                            nc.tensor.matmul(
                                ps1[:, q * 512:(q + 1) * 512],
                                Rsb[:, (h * 2) * 128:(h * 2 + 1) * 128],
                                xk[0][:, g * 512:(g + 1) * 512],
                                start=True, stop=False,
                            )
                            nc.tensor.matmul(
                                ps1[:, q * 512:(q + 1) * 512],
                                Rsb[:, (h * 2 + 1) * 128:(h * 2 + 2) * 128],
                                xk[1][:, g * 512:(g + 1) * 512],
                                start=False, stop=True,
                            )
                        # ps1 [s, (b16, j64)] -> out1 [s, b, h*64+j]
                        copy_eng(nck)(
                            out1v[:, g2 * 16:(g2 + 1) * 16, h * 64:(h + 1) * 64],
                            ps1.rearrange("p (b j) -> p b j", b=16),
                        )
                        nck += 1

                # ---- permute-1: out1 [s,(b,hj)] -> X2 [hj,(s,b)] --------
                X2 = bigpool.tile([128, 128 * PB], BF16, tag="X2")
                X2v = X2.rearrange("p (s b) -> p s b", b=PB)  # [p, s, b] view
                for t8 in range(PB // 8):
                    psT = pspool.tile([128, 1024], F32, tag="ps")
                    for q in range(8):
                        bb = t8 * 8 + q
                        nc.tensor.matmul(
                            psT[:, q * 128:(q + 1) * 128],
                            out1[:, bb * 128:(bb + 1) * 128], Isb[:, :],
                            start=True, stop=True,
                        )
                    # psT [hj, (b8, s128)] -> X2 cols s*PB + b (16B chunks)
                    copy_eng(t8)(
                        X2v[:, :, t8 * 8:(t8 + 1) * 8],
                        psT.rearrange("p (b s) -> p s b", b=8),
                    )

                # ---------------- stage 2: mix blocks per slot -----------
                # O2 [m, (b, s)]: col = b*128 + s  (shares buffer with out1)
                O2 = bigpool.tile([128, PB * 128], BF16, tag="big1")
                O2v = O2.rearrange("p (b s) -> p b s", s=128)
                for g8 in range(16):  # 8 slots per 2-bank PSUM tile
                    ps2 = pspool.tile([128, 1024], F32, tag="ps")
                    for q in range(8):
                        s = 8 * g8 + q
                        nc.tensor.matmul(
                            ps2[:, q * 128:(q + 1) * 128],
                            Gsb[:, s * 128:(s + 1) * 128],
                            X2[:, s * PB:(s + 1) * PB],
                            start=True, stop=True,
                        )
                    # ps2 [m, (s8, b128)] -> O2 cols b*128 + s (16B chunks)
                    copy_eng(g8 + 1)(
                        O2v[:, :, g8 * 8:(g8 + 1) * 8],
                        ps2.rearrange("p (s b) -> p b s", s=8),
                    )

                # ---- permute-2: O2 [m,(b,s)] -> T2 [s,(m,b)] -------------
                T2 = bigpool.tile([128, 128 * PB], BF16, tag="T2")
                T2v = T2.rearrange("p (m b) -> p m b", b=PB)
                for t8 in range(PB // 8):
                    psT = pspool.tile([128, 1024], F32, tag="ps")
                    for q in range(8):
                        bb = t8 * 8 + q
                        nc.tensor.matmul(
                            psT[:, q * 128:(q + 1) * 128],
                            O2[:, bb * 128:(bb + 1) * 128], Isb[:, :],
                            start=True, stop=True,
                        )
                    # psT [s, (b8, m128)] -> T2 cols m*PB + b (16B chunks)
                    copy_eng(t8 + 1)(
                        T2v[:, :, t8 * 8:(t8 + 1) * 8],
                        psT.rearrange("p (b m) -> p m b", b=8),
                    )

                # ---------- stage 3: IDFT per output block ---------------
                for g8 in range(8):  # 8 output blocks i -> one [128,2048] store
                    yt = wpool.tile([128, 2048], BF16, tag="yt", name="yt")
                    for half in range(2):  # 4 blocks i per 2-bank PSUM tile
                        ps3 = pspool.tile([128, 1024], F32, tag="ps")
                        for q in range(4):
                            i = g8 * 8 + half * 4 + q
                            nc.tensor.matmul(
                                ps3[:, q * 256:(q + 1) * 256],
                                T2[:, i * PB:(i + 1) * PB],
                                Risb[:, 0:256], start=True, stop=False,
                            )
                            nc.tensor.matmul(
                                ps3[:, q * 256:(q + 1) * 256],
                                T2[:, (64 + i) * PB:(65 + i) * PB],
                                Risb[:, 256:512], start=False, stop=True,
                            )
                        copy_eng(half + g8)(
                            yt[:, half * 1024:(half + 1) * 1024], ps3[:, :]
                        )
                    nc.gpsimd.dma_start(
                        Y.ap()[p * PB:(p + 1) * PB, g8 * 2048:(g8 + 1) * 2048],
                        yt[:, :],
                    )
    nc.compile()
    return nc


def _get_nc():
    if "nc" not in _NC_CACHE:
        _NC_CACHE["nc"] = _build_nc()
    return _NC_CACHE["nc"]


def run(x, W_real, W_imag, trace=False):
    x = np.asarray(x, dtype=np.float32)
    Wr = np.asarray(W_real, dtype=np.float32)
    Wi = np.asarray(W_imag, dtype=np.float32)

    nc = _get_nc()
    R, Ri, ident = _build_consts()
    G = _build_g(Wr, Wi)

    in_maps = []
    for c in range(NCORES):
        xc = x[c * BC:(c + 1) * BC]                       # [256, 16384]
        # -> [t, b, j] -> [pass, t(256), b(128), j(64)]
        xcp = xc.reshape(BC, KIN, BLOCK).transpose(2, 0, 1)
        xcp = xcp.reshape(BLOCK, NPASS, PB, KIN).transpose(1, 0, 2, 3)
        in_maps.append({
            "xP": np.ascontiguousarray(xcp).astype(NPBF16),
            "G": G, "R": R, "Ri": Ri, "Ident": ident,
        })
    res = run_bass_kernel_spmd(
        nc, in_maps, core_ids=list(range(NCORES)), trace=trace
    )
    out = np.concatenate([r["Y"] for r in res.results], axis=0)
    return np.ascontiguousarray(out.astype(np.float32)), res


def kernel(x, W_real, W_imag):
    out, _ = run(x, W_real, W_imag)
    return out


# revision 8
# speedup vs baseline: 1.6289x; 1.0038x over previous
"""Block-FFT circulant matmul (BlockFFTDirectPrior) as a Trainium2 Bass kernel.

Math: out = ifft( einsum('bjf,ijf->bif', fft(x_blocks), conj(W_full)) ).real
with 64x64 blocks of size 256, batch 2048.

Everything is real-matmul based (no complex arithmetic, no FFT butterflies):
  stage 1: per input block j, spectrum = x_j @ R            (DFT as matmul)
  stage 2: per frequency slot s, mix blocks j -> i with a 128x128 real
           matrix G_s built from W_real/W_imag (re/im packed)
  stage 3: per output block i, time = spectrum_i @ Rinv     (IDFT as matmul)

Spectrum packing (256 real values per block): half A = Re f=0..127,
half B = [Re f=128, Im f=1..127].  Slot s pairs (A[s], B[s]): slot 0
carries DC/Nyquist (both real), slots 1..127 carry complex bin f=s.

Between stages the partition axis must rotate (spec -> block -> spec).
Both permutes are done as PE transposes: matmul(lhsT=data, rhs=identity).

Layouts keep every PE operand (stationary and moving) contiguous in SBUF;
the PSUM->SBUF drain copies absorb the reorders (strided 4B PSUM reads,
16B-chunk SBUF writes):
  out1 [s | b, hj]   (stage-1 out; b-major so T1 lhsT tiles contiguous)
  X2   [hj | s, b]   (s-major so stage-2 moving operand contiguous)
  O2   [m | b, s]    (b-major so T2 lhsT tiles contiguous)
  T2sb [s | m, b]    (m-major so stage-3 lhsT tiles contiguous)

G (4MB bf16) stays resident in SBUF.  Output is written bf16 and upcast
to fp32 on the host during the unshard (halves the output DMA).  x chunks
for pass p+1 prefetch (double-buffered) while pass p computes; ~50 warmup
matmuls run during the initial x DMA wait so the PE HAM clock is at 2.4GHz
when stage 1 starts.  The three big SBUF intermediates rotate through 3
buffers (interval coloring).

Sharding: data-parallel over batch across 8 NeuronCores (256 rows each),
2 passes of 128 rows per core.  All matmul operands are bf16.
"""

import numpy as np
import ml_dtypes

import concourse.bass as bass
import concourse.mybir as mybir
from concourse import bacc
from concourse.tile import TileContext
from concourse.bass_utils import run_bass_kernel_spmd

B, KIN, KOUT, BLOCK = 2048, 64, 64, 256
NCORES = 8
BC = B // NCORES            # 256 batch rows per core
NPASS = 2
PB = BC // NPASS            # 128 batch rows per pass

F32 = mybir.dt.float32
BF16 = mybir.dt.bfloat16
NPBF16 = ml_dtypes.bfloat16

_NC_CACHE = {}


def _build_consts():
    """DFT / inverse-DFT matrices + identity, bf16, kernel layouts."""
    t = np.arange(BLOCK)
    f = np.arange(128)
    ang = 2.0 * np.pi * np.outer(t, f) / BLOCK          # [t, f]
    RA = np.cos(ang)                                    # re f=0..127
    RB = -np.sin(ang)                                   # im f=1..127
    RB[:, 0] = np.cos(np.pi * t)                        # re f=128 in col 0
    R = np.zeros((2, 2, 128, 128), dtype=NPBF16)        # [h, kt, t(128), m]
    for kt in range(2):
        R[0, kt] = RA[kt * 128:(kt + 1) * 128, :].astype(NPBF16)
        R[1, kt] = RB[kt * 128:(kt + 1) * 128, :].astype(NPBF16)

    s = np.arange(128)
    tp = np.arange(BLOCK)
    angi = 2.0 * np.pi * np.outer(s, tp) / BLOCK        # [s, t']
    w = np.full((128, 1), 2.0 / BLOCK)
    w[0] = 1.0 / BLOCK
    RiA = w * np.cos(angi)
    RiB = -(2.0 / BLOCK) * np.sin(angi)
    RiB[0, :] = (1.0 / BLOCK) * np.cos(np.pi * tp)      # Nyquist (real) term
    Ri = np.stack([RiA, RiB]).astype(NPBF16)            # [2, 128, 256]

    ident = np.eye(128, dtype=NPBF16)
    # pack for single-DMA loads: R4 [t(128), (h,kt,m)=512], Ri2 [s(128), (h,t')=512]
    R4 = np.ascontiguousarray(R.transpose(2, 0, 1, 3).reshape(128, 512))
    Ri2 = np.ascontiguousarray(Ri.transpose(1, 0, 2).reshape(128, 512))
    return R4, Ri2, ident


def _build_g(Wr, Wi):
    """Stage-2 mixing matrices, layout [k=(h*64+j), s, m=(re_i|im_i)], bf16."""
    G = np.zeros((128, 128, 128), dtype=np.float32)     # [s, k, m]
    G[0, :64, :64] = Wr[:, :, 0].T
    G[0, 64:, 64:] = Wr[:, :, 128].T
    WrT = np.transpose(Wr, (2, 1, 0))                   # [f, j, i]
    WiT = np.transpose(Wi, (2, 1, 0))
    G[1:, :64, :64] = WrT[1:128]
    G[1:, :64, 64:] = -WiT[1:128]
    G[1:, 64:, :64] = WiT[1:128]
    G[1:, 64:, 64:] = WrT[1:128]
    return np.ascontiguousarray(G.transpose(1, 0, 2)).astype(NPBF16)


def _build_nc():
    nc = bacc.Bacc("TRN2", target_bir_lowering=False, debug=False)
    # xP layout [pass, t(256), b(128), j(64)]
    xP = nc.dram_tensor("xP", [NPASS, BLOCK, PB, KIN], BF16, kind="ExternalInput")
    Gt = nc.dram_tensor("G", [128, 128, 128], BF16, kind="ExternalInput")
    Rt = nc.dram_tensor("R", [128, 512], BF16, kind="ExternalInput")
    Rit = nc.dram_tensor("Ri", [128, 512], BF16, kind="ExternalInput")
    It = nc.dram_tensor("Ident", [128, 128], BF16, kind="ExternalInput")
    Y = nc.dram_tensor("Y", [BC, KOUT * BLOCK], BF16, kind="ExternalOutput")

    def copy_eng(k):
        return nc.vector.tensor_copy if k % 2 == 0 else nc.scalar.copy

    with TileContext(nc) as tc:
        with (
            tc.tile_pool(name="const", bufs=1) as cpool,
            tc.tile_pool(name="big", bufs=1) as bigpool,
            tc.tile_pool(name="xkp", bufs=2) as xkpool,
            tc.tile_pool(name="work", bufs=3) as wpool,
            tc.tile_pool(name="ps", bufs=4, space="PSUM") as pspool,
        ):
            # R first on the sync queue (needed by the first stage-1 MM,
            # tiny), then the pass-0 x chunks follow immediately behind it.
            Rsb = cpool.tile([128, 4 * 128], BF16)
            nc.sync.dma_start(Rsb[:, :], Rt.ap())
            # Ri / I on the scalar HWDGE queue, G on gpsimd SWDGE: keeps the
            # sync queue free for the x input stream.
            Risb = cpool.tile([128, 512], BF16)
            nc.scalar.dma_start(Risb[:, :], Rit.ap())
            Isb = cpool.tile([128, 128], BF16)
            nc.scalar.dma_start(Isb[:, :], It.ap())
            # G resident: [k | s, m], col = s*128 + m
            Gsb = cpool.tile([128, 128 * 128], BF16)
            for c in range(4):
                nc.gpsimd.dma_start(
                    Gsb[:, c * 4096:(c + 1) * 4096],
                    Gt.ap()[:, c * 32:(c + 1) * 32],
                )

            def load_xk(p):
                """x chunks for pass p: xk[kt] [t(128), (b,j)], double-buffered."""
                xk = [
                    xkpool.tile([128, PB * KIN], BF16, tag=f"xk{kt}",
                                name=f"xk{kt}")
                    for kt in range(2)
                ]
                for q in range(4):      # b-chunk outer, kt inner: MM order
                    for kt in range(2):
                        nc.sync.dma_start(
                            xk[kt][:, q * 2048:(q + 1) * 2048],
                            xP.ap()[p, kt * 128:(kt + 1) * 128,
                                    q * 32:(q + 1) * 32],
                        )
                return xk

            # HAM warmup: keep the PE busy during the initial x DMA wait so
            # stage 1 starts at 2.4GHz.  Results are discarded.
            ps_w = pspool.tile([128, 1024], F32, tag="ps")
            for w in range(48):
                nc.tensor.matmul(
                    ps_w[:, (w % 8) * 128:(w % 8 + 1) * 128],
                    Rsb[:, 0:128], Rsb[:, 128:256],
                    start=True, stop=True,
                )

            def stage1(xk, tag):
                """DFT per block: xk [t(128),(b,j)] -> out1 [s,(b,hj)]."""
                out1 = bigpool.tile([128, 2 * PB * KIN], BF16, tag=tag)
                out1v = out1.rearrange("p (b hj) -> p b hj", hj=128)
                nck = 0
                for g2 in range(8):  # 2-bank PSUM tiles, 2 chunks each
                    for h in range(2):
                        ps1 = pspool.tile([128, 1024], F32, tag="ps")
                        for q in range(2):
                            g = g2 * 2 + q
                            nc.tensor.matmul(
                                ps1[:, q * 512:(q + 1) * 512],
                                Rsb[:, (h * 2) * 128:(h * 2 + 1) * 128],
                                xk[0][:, g * 512:(g + 1) * 512],
                                start=True, stop=False,
                            )
                            nc.tensor.matmul(
                                ps1[:, q * 512:(q + 1) * 512],
                                Rsb[:, (h * 2 + 1) * 128:(h * 2 + 2) * 128],
                                xk[1][:, g * 512:(g + 1) * 512],
                                start=False, stop=True,
                            )
                        # ps1 [s, (b16, j64)] -> out1 [s, b, h*64+j]
                        copy_eng(nck)(
                            out1v[:, g2 * 16:(g2 + 1) * 16, h * 64:(h + 1) * 64],
                            ps1.rearrange("p (b j) -> p b j", b=16),
                        )
                        nck += 1
                return out1

            def permute1(out1, tag):
                """out1 [s,(b,hj)] -> X2 [hj,(s,b)] via PE transposes."""
                X2 = bigpool.tile([128, 128 * PB], BF16, tag=tag)
                X2v = X2.rearrange("p (s b) -> p s b", b=PB)
                for t8 in range(PB // 8):
                    psT = pspool.tile([128, 1024], F32, tag="ps")
                    for q in range(8):
                        bb = t8 * 8 + q
                        nc.tensor.matmul(
                            psT[:, q * 128:(q + 1) * 128],
                            out1[:, bb * 128:(bb + 1) * 128], Isb[:, :],
                            start=True, stop=True,
                        )
                    # psT [hj, (b8, s128)] -> X2 cols s*PB + b (16B chunks)
                    copy_eng(t8)(
                        X2v[:, :, t8 * 8:(t8 + 1) * 8],
                        psT.rearrange("p (b s) -> p s b", b=8),
                    )
                return X2

            def stage2(X2, tag):
                """Mix blocks per slot: O2 [m, (b, s)], col = b*128 + s."""
                O2 = bigpool.tile([128, PB * 128], BF16, tag=tag)
                O2v = O2.rearrange("p (b s) -> p b s", s=128)
                for g8 in range(16):  # 8 slots per 2-bank PSUM tile
                    ps2 = pspool.tile([128, 1024], F32, tag="ps")
                    for q in range(8):
                        s = 8 * g8 + q
                        nc.tensor.matmul(
                            ps2[:, q * 128:(q + 1) * 128],
                            Gsb[:, s * 128:(s + 1) * 128],
                            X2[:, s * PB:(s + 1) * PB],
                            start=True, stop=True,
                        )
                    # ps2 [m, (s8, b128)] -> O2 cols b*128 + s (16B chunks)
                    copy_eng(g8 + 1)(
                        O2v[:, :, g8 * 8:(g8 + 1) * 8],
                        ps2.rearrange("p (s b) -> p b s", s=8),
                    )
                return O2

            def permute2(O2, tag):
                """O2 [m,(b,s)] -> T2sb [s,(m,b)] via PE transposes."""
                T2 = bigpool.tile([128, 128 * PB], BF16, tag=tag)
                T2v = T2.rearrange("p (m b) -> p m b", b=PB)
                for t8 in range(PB // 8):
                    psT = pspool.tile([128, 1024], F32, tag="ps")
                    for q in range(8):
                        bb = t8 * 8 + q
                        nc.tensor.matmul(
                            psT[:, q * 128:(q + 1) * 128],
                            O2[:, bb * 128:(bb + 1) * 128], Isb[:, :],
                            start=True, stop=True,
                        )
                    # psT [s, (b8, m128)] -> T2 cols m*PB + b (16B chunks)
                    copy_eng(t8 + 1)(
                        T2v[:, :, t8 * 8:(t8 + 1) * 8],
                        psT.rearrange("p (b m) -> p m b", b=8),
                    )
                return T2

            def stage3(p, T2):
                """IDFT per output block i; yt bf16 -> Y."""
                for g8 in range(8):
                    yt = wpool.tile([128, 2048], BF16, tag="yt", name="yt")
                    for half in range(2):  # 4 blocks i per 2-bank PSUM tile
                        ps3 = pspool.tile([128, 1024], F32, tag="ps")
                        for q in range(4):
                            i = g8 * 8 + half * 4 + q
                            nc.tensor.matmul(
                                ps3[:, q * 256:(q + 1) * 256],
                                T2[:, i * PB:(i + 1) * PB],
                                Risb[:, 0:256], start=True, stop=False,
                            )
                            nc.tensor.matmul(
                                ps3[:, q * 256:(q + 1) * 256],
                                T2[:, (64 + i) * PB:(65 + i) * PB],
                                Risb[:, 256:512], start=False, stop=True,
                            )
                        copy_eng(half + g8)(
                            yt[:, half * 1024:(half + 1) * 1024], ps3[:, :]
                        )
                    nc.gpsimd.dma_start(
                        Y.ap()[p * PB:(p + 1) * PB, g8 * 2048:(g8 + 1) * 2048],
                        yt[:, :],
                    )

            # big intermediates rotate through 3 buffers (interval coloring):
            #   big1 = {out1_0, T2_0}, big2 = {X2_0, out1_1, O2_1},
            #   big3 = {O2_0, X2_1, T2_1}
            xk0 = load_xk(0)
            o1_0 = stage1(xk0, "big1")
            xk1 = load_xk(1)              # prefetch during pass-0 compute
            x2_0 = permute1(o1_0, "big2")
            o2_0 = stage2(x2_0, "big3")
            t2_0 = permute2(o2_0, "big1")
            stage3(0, t2_0)
            o1_1 = stage1(xk1, "big2")
            x2_1 = permute1(o1_1, "big3")
            o2_1 = stage2(x2_1, "big2")
            t2_1 = permute2(o2_1, "big3")
            stage3(1, t2_1)
    nc.compile()
    return nc


def _get_nc():
    if "nc" not in _NC_CACHE:
        _NC_CACHE["nc"] = _build_nc()
    return _NC_CACHE["nc"]


def run(x, W_real, W_imag, trace=False):
    x = np.asarray(x, dtype=np.float32)
    Wr = np.asarray(W_real, dtype=np.float32)
    Wi = np.asarray(W_imag, dtype=np.float32)

    nc = _get_nc()
    R, Ri, ident = _build_consts()
    G = _build_g(Wr, Wi)

    in_maps = []
    for c in range(NCORES):
        xc = x[c * BC:(c + 1) * BC]                       # [256, 16384]
        # -> [t, b, j] -> [pass, t(256), b(128), j(64)]
        xcp = xc.reshape(BC, KIN, BLOCK).transpose(2, 0, 1)
        xcp = xcp.reshape(BLOCK, NPASS, PB, KIN).transpose(1, 0, 2, 3)
        in_maps.append({
            "xP": np.ascontiguousarray(xcp).astype(NPBF16),
            "G": G, "R": R, "Ri": Ri, "Ident": ident,
        })
    res = run_bass_kernel_spmd(
        nc, in_maps, core_ids=list(range(NCORES)), trace=trace
    )
    out = np.concatenate([r["Y"] for r in res.results], axis=0)
    return np.ascontiguousarray(out.astype(np.float32)), res


def kernel(x, W_real, W_imag):
    out, _ = run(x, W_real, W_imag)
    return out
